# revision 20
# baseline (speedup 1.0000x reference)
"""Trainium2 Bass kernel for the 4-step shift-only MAF (MADE) chain.

Strategy: tensor-parallel over the hidden/feature dims across 8 NeuronCores
(column-parallel for every layer), with activations kept transposed
[features, batch] so matmuls chain without transposes.  The inter-step
`z[:, ::-1]` permute is folded into the host-side weight prep (W0 rows /
W3 cols reversed for odd steps), so the device never flips.  After each
layer an AllGather (partition-axis concat) rebuilds the full activation.

v2 perf changes vs the plain bf16 baseline:
  - fp8e4 (e4m3) weights AND activations, x16 scaling on both (values
    stay in normal range; relu is positively homogeneous so the scale
    folds into the activation instruction's scale/bias).  Matmuls run
    in DoubleRow perf mode (2 k-tiles per instruction, 2x PE rate).
  - a dummy 0-dep AllGather issued at kernel start pulls the one-time
    collective entry barrier off the critical path (it overlaps the
    initial weight DMA + L0 compute instead of stalling the first real
    AllGather).
  - AllGather bounce traffic is spread across engine queues: gather
    results are read back per-rank (8 small contiguous DMAs on
    sync/vector instead of one big strided rearrange on sync), so
    matmul k-pairs chase individual rank landings via subtile deps;
    weight prefetch lives on gpsimd's queue; AG-input bounce writes on
    scalar's queue right behind the relu that produces them.

Device per-core program (SPMD, identical for all cores; per-core data
arrives via in_maps):
  z_loc [128,100] (f32, x16) and full zT [128,8,100] (fp8, x16) start
  as x.  Per step s: h0 = relu(W0e[s].T @ z) (2 psum m-tiles) -> AG ->
  h1 -> AG -> h2 -> AG -> shift = W3e[s].T @ h2; z_loc -= shift + b3;
  AG z (not on last step).  Finally out = ones.T @ (z_loc^2) per core
  -> [1,100]; host sums the 8 partials, divides by the scale^2, and
  adds the log(2pi) constant.
"""

import os
import sys

import numpy as np

for _p in ("/opt/trn_rl_repo", "/opt/trn_rl_repo/concourse"):
    if _p not in sys.path:
        sys.path.insert(0, _p)

B = 100
DIM = 1024
H = 2048
STEPS = 4
NC = 8
P = 128
KD = DIM // P   # 8 z k-tiles
KH = H // P     # 16 h k-tiles
MH = 2          # h m-tiles per core (256 local cols)
HL = H // NC    # 256
DL = DIM // NC  # 128
LOG_2PI = float(np.log(2.0 * np.pi))
F32 = np.float32

# compute dtype for weights / gathered activations ("float8e4" | "bfloat16")
WDTYPE = os.environ.get("MAF_WDTYPE", "float8e4")
SCALE = 16.0 if WDTYPE == "float8e4" else 1.0  # both weight and act scale


def _np_wdt():
    from concourse import mybir
    if WDTYPE == "float8e4":
        return mybir.dt.np(mybir.dt.float8e4)
    from ml_dtypes import bfloat16
    return bfloat16


def _made_mask(n_in, n_out, exclusive):
    d_in, d_out = n_in // DIM, n_out // DIM
    deg_in = np.arange(n_in) // d_in
    deg_out = np.arange(n_out) // d_out
    if exclusive:
        m = deg_out[None, :] > deg_in[:, None]
    else:
        m = deg_out[None, :] >= deg_in[:, None]
    return m.astype(F32)


def _prep_inputs(x, W0, b0, W1, b1, W2, b2, W3, b3):
    """Host-side: mask, fold flips, shard, scale, pre-arrange into SBUF
    layouts.  Returns in_maps: list of dicts, one per core."""
    M0 = _made_mask(DIM, H, True)
    M1 = _made_mask(H, H, False)
    M3 = _made_mask(H, DIM, False)

    xT = np.ascontiguousarray(x.T.astype(F32))              # [1024, 100]
    xt_arr = np.ascontiguousarray(
        xT.reshape(KD, P, B).transpose(1, 0, 2))            # [128, 8, 100]

    # Per-step effective (masked + flip-folded) weights
    W0e, W1e, W2e, W3e, b3e = [], [], [], [], []
    for s in range(STEPS):
        w0 = W0[s] * M0
        if s % 2 == 1:
            w0 = w0[::-1, :]
        w3 = W3[s] * M3
        b3s = b3[s]
        if s % 2 == 1:
            w3 = w3[:, ::-1]
            b3s = b3s[::-1]
        W0e.append(w0)
        W1e.append(W1[s] * M1)
        W2e.append(W2[s] * M1)
        W3e.append(w3)
        b3e.append(b3s)

    wdt = _np_wdt()
    S = SCALE
    in_maps = []
    for c in range(NC):
        hc = slice(HL * c, HL * (c + 1))
        dc = slice(DL * c, DL * (c + 1))
        w0c = np.stack([
            (W0e[s][:, hc] * S).reshape(KD, P, MH, P).transpose(1, 0, 2, 3)
            for s in range(STEPS)])                          # [4,128,8,2,128]
        w1c = np.stack([
            (W1e[s][:, hc] * S).reshape(KH, P, MH, P).transpose(1, 0, 2, 3)
            for s in range(STEPS)])                          # [4,128,16,2,128]
        w2c = np.stack([
            (W2e[s][:, hc] * S).reshape(KH, P, MH, P).transpose(1, 0, 2, 3)
            for s in range(STEPS)])
        w3c = np.stack([
            (W3e[s][:, dc] * S).reshape(KH, P, P).transpose(1, 0, 2)
            for s in range(STEPS)])                          # [4,128,16,128]
        # biases for relu layers are applied in the x{S} activation domain;
        # all four packed into one [P, 7] tensor per step (single DMA):
        # cols 0:2 = b0 (m0,m1), 2:4 = b1, 4:6 = b2, 6 = b3.
        ball = np.stack([
            np.concatenate([
                (b0[s][hc] * S).reshape(MH, P).T,
                (b1[s][hc] * S).reshape(MH, P).T,
                (b2[s][hc] * S).reshape(MH, P).T,
                (b3e[s][dc] * S).reshape(1, P).T,
            ], axis=1)
            for s in range(STEPS)])                          # [4, 128, 7]
        in_maps.append({
            "xt": np.ascontiguousarray((xt_arr * S).astype(wdt)),
            "xloc": np.ascontiguousarray(xT[dc, :] * S),     # [128, 100] f32
            "w0": np.ascontiguousarray(w0c.astype(wdt)),
            "w1": np.ascontiguousarray(w1c.astype(wdt)),
            "w2": np.ascontiguousarray(w2c.astype(wdt)),
            "w3": np.ascontiguousarray(w3c.astype(wdt)),
            "ball": np.ascontiguousarray(ball.astype(F32)),
        })
    return in_maps


_CACHED_NC = {}


def _build_module(repeat=1):
    """Build the SPMD module. repeat>1 runs the whole MAF body N times
    back-to-back (timing builds only; output is then meaningless)."""
    if repeat in _CACHED_NC:
        return _CACHED_NC[repeat]

    from concourse import bass, bacc, tile, mybir

    f32 = mybir.dt.float32
    is_fp8 = WDTYPE == "float8e4"
    wdt = mybir.dt.float8e4 if is_fp8 else mybir.dt.bfloat16
    KS = 2 if is_fp8 else 1          # k-tiles consumed per matmul
    PM = mybir.MatmulPerfMode.DoubleRow if is_fp8 else None
    INV_S = 1.0 / SCALE
    Relu = mybir.ActivationFunctionType.Relu
    Ident = mybir.ActivationFunctionType.Identity
    Square = mybir.ActivationFunctionType.Square
    RG = [list(range(NC))]
    no_cc = bool(int(os.environ.get("MAF_NO_CC", "0")))    # timing ablation
    # opt-in: a 0-dep warmup collective.  Measured on the axon pool it
    # LOSES ~15us (the CC stream serializes barrier -> dummy -> real AG).
    use_dummy = bool(int(os.environ.get("MAF_DUMMY", "0")))

    nc = bacc.Bacc("TRN2", target_bir_lowering=False, debug=False,
                   num_devices=NC)

    xt_d = nc.dram_tensor("xt", [P, KD, B], wdt, kind="ExternalInput")
    xloc_d = nc.dram_tensor("xloc", [P, B], f32, kind="ExternalInput")
    w0_d = nc.dram_tensor("w0", [STEPS, P, KD, MH, P], wdt, kind="ExternalInput")
    w1_d = nc.dram_tensor("w1", [STEPS, P, KH, MH, P], wdt, kind="ExternalInput")
    w2_d = nc.dram_tensor("w2", [STEPS, P, KH, MH, P], wdt, kind="ExternalInput")
    w3_d = nc.dram_tensor("w3", [STEPS, P, KH, P], wdt, kind="ExternalInput")
    ball_d = nc.dram_tensor("ball", [STEPS, P, 3 * MH + 1], f32,
                            kind="ExternalInput")
    sq_d = nc.dram_tensor("sq", [1, B], f32, kind="ExternalOutput")

    trace_sim = bool(int(os.environ.get("MAF_TRACE_SIM", "0")))
    with tile.TileContext(nc, trace_sim=trace_sim) as tc:
        with (
            # bufs=4: all four steps' weights prefetch at kernel start
            # (during the collective entry barrier, while HBM is idle),
            # so early-step bounce receipts don't contend with weight
            # streaming.  ~44KB/partition of SBUF, well within budget.
            tc.tile_pool(name="w01", bufs=4) as wpool,
            tc.tile_pool(name="hf", bufs=2) as hpool,
            tc.tile_pool(name="zp", bufs=2) as zpool,
            tc.tile_pool(name="loc", bufs=2) as locpool,
            tc.tile_pool(name="bia", bufs=4) as bpool,
            tc.tile_pool(name="cst", bufs=1) as cpool,
            tc.tile_pool(name="ps", bufs=4, space=bass.MemorySpace.PSUM) as pspool,
            tc.tile_pool(name="drb", bufs=2, space="DRAM") as dpool,
        ):
            if use_dummy and not no_cc:
                # 0-dependency warmup collective: absorbs the one-time
                # entry barrier while weights stream in.
                dmi = cpool.tile([P, 4], mybir.dt.int8, tag="dmi")
                nc.gpsimd.memset(dmi[:], 0)
                dum_in = dpool.tile([P, 4], mybir.dt.int8, tag="dmin", bufs=1)
                nc.gpsimd.dma_start(dum_in[:], dmi[:])
                dum_out = dpool.tile([NC, P, 4], mybir.dt.int8, tag="dmout",
                                     bufs=1)
                nc.gpsimd.collective_compute(
                    "AllGather", mybir.AluOpType.bypass, replica_groups=RG,
                    ins=[dum_in.opt()], outs=[dum_out.opt()])

            ones = cpool.tile([P, 1], f32, tag="ones")
            nc.gpsimd.memset(ones[:], 1.0)

            zT = zpool.tile([P, KD, B], wdt, tag="zT")
            nc.sync.dma_start(zT[:], xt_d[:])  # xt pre-arranged [p, c, b]
            zloc = zpool.tile([P, B], f32, tag="zloc")
            nc.sync.dma_start(zloc[:], xloc_d[:])

            def h_layer(w_t, b_t, rhsT, n_k, out_tag):
                """col-parallel hidden layer + AG; returns full hT tile."""
                kp = n_k // KS
                if len(rhsT.shape) == 4:
                    # hT [P, NC, MH, B]: pair j == rank j's block (fp8),
                    # or single k-tile (k//MH, k%MH) in bf16 mode.
                    if KS == 2:
                        rhs_j = lambda j: rhsT[:, j, :, :]
                    else:
                        rhs_j = lambda j: rhsT[:, j // MH, j % MH, :]
                else:
                    # zT [P, KD, B]
                    if KS == 2:
                        rhs_j = lambda j: rhsT[:, 2 * j:2 * j + 2, :]
                    else:
                        rhs_j = lambda j: rhsT[:, j, :]
                hloc = locpool.tile([P, MH, B], wdt, tag="hloc")
                agi = dpool.tile([P, MH, B], wdt, tag="agi")
                for m in range(MH):
                    ps = pspool.tile([P, B], f32, tag="ps")
                    for j in range(kp):
                        if KS == 2:
                            w_ap = w_t[:, 2 * j:2 * j + 2, m, :]
                        else:
                            w_ap = w_t[:, j, m, :]
                        nc.tensor.matmul(
                            ps[:], w_ap, rhs_j(j),
                            start=(j == 0), stop=(j == kp - 1), perf_mode=PM)
                    nc.scalar.activation(hloc[:, m, :], ps[:], Relu,
                                         bias=b_t[:, m:m + 1], scale=INV_S)
                    # bounce write per m-tile on SEPARATE queues: m0's HBM
                    # write+receipt overlaps m1's matmuls+relu, and m1's
                    # receipt (which gates the AG trigger) doesn't queue
                    # behind m0's on the same HWDGE ring.
                    weng = nc.scalar if m == 0 else nc.sync
                    weng.dma_start(agi[:, m, :], hloc[:, m, :],
                                   single_packet=True)
                ago = dpool.tile([NC, P, MH, B], wdt, tag="ago",
                                 addr_space="Shared")
                hT = hpool.tile([P, NC, MH, B], wdt, tag=out_tag)
                if no_cc:
                    nc.sync.dma_start(ago[0, :, :, :], agi[:])
                else:
                    nc.gpsimd.collective_compute(
                        "AllGather", mybir.AluOpType.bypass, replica_groups=RG,
                        ins=[agi.opt()], outs=[ago.opt()])
                # rank-pair contiguous readback spread over all three DMA
                # queues (gpsimd is idle here since Round D prefetches all
                # weights at t=0); matmul k-pairs chase individual pair
                # landings via subtile deps and the tail receipt lands
                # earlier with three rings draining in parallel.
                rb_eng = (nc.sync, nc.scalar, nc.gpsimd, nc.sync)
                for i, c in enumerate(range(0, NC, 2)):
                    rb_eng[i].dma_start(
                        hT[:, c:c + 2, :, :],
                        ago[c:c + 2].rearrange("c p m b -> p c m b"))
                return hT

            for it in range(STEPS * repeat):
                s = it % STEPS
                is_last = it == STEPS * repeat - 1
                w0t = wpool.tile([P, KD, MH, P], wdt, tag="w0")
                nc.gpsimd.dma_start(w0t[:], w0_d[s])
                w1t = wpool.tile([P, KH, MH, P], wdt, tag="w1")
                nc.gpsimd.dma_start(w1t[:], w1_d[s])
                w2t = wpool.tile([P, KH, MH, P], wdt, tag="w2")
                nc.gpsimd.dma_start(w2t[:], w2_d[s])
                w3t = wpool.tile([P, KH, P], wdt, tag="w3")
                nc.gpsimd.dma_start(w3t[:], w3_d[s])
                ballt = bpool.tile([P, 3 * MH + 1], f32, tag="ball")
                nc.gpsimd.dma_start(ballt[:], ball_d[s])
                b0t, b1t, b2t = (ballt[:, 2 * i:2 * i + MH] for i in range(3))
                b3t = ballt[:, 3 * MH:3 * MH + 1]

                h0T = h_layer(w0t, b0t, zT, KD, "h0T")
                h1T = h_layer(w1t, b1t, h0T, KH, "h1T")
                h2T = h_layer(w2t, b2t, h1T, KH, "h2T")

                ps3 = pspool.tile([P, B], f32, tag="ps")
                for j in range(KH // KS):
                    if KS == 2:
                        nc.tensor.matmul(ps3[:], w3t[:, 2 * j:2 * j + 2, :],
                                         h2T[:, j, :, :],
                                         start=(j == 0),
                                         stop=(j == KH // KS - 1),
                                         perf_mode=PM)
                    else:
                        nc.tensor.matmul(ps3[:], w3t[:, j, :],
                                         h2T[:, j // MH, j % MH, :],
                                         start=(j == 0),
                                         stop=(j == KH - 1))
                # sh = shift*S + b3*S (still in the xS domain)
                sh = locpool.tile([P, B], f32, tag="sh")
                nc.scalar.activation(sh[:], ps3[:], Ident,
                                     bias=b3t[:, 0:1], scale=INV_S)
                if not is_last:
                    # fp8 AG input first (critical path), f32 update after
                    # (overlaps the collective).
                    zlb = locpool.tile([P, B], wdt, tag="zlb")
                    nc.vector.tensor_sub(zlb[:], zloc[:], sh[:])
                zloc2 = zpool.tile([P, B], f32, tag="zloc")
                nc.vector.tensor_sub(zloc2[:], zloc[:], sh[:])
                zloc = zloc2

                if not is_last:
                    zin = dpool.tile([P, B], wdt, tag="zin")
                    # sync queue: issues right behind the vector sub that
                    # produces zlb, without queuing behind scalar's acts.
                    nc.sync.dma_start(zin[:], zlb[:], single_packet=True)
                    zout = dpool.tile([NC, P, B], wdt, tag="zout",
                                      addr_space="Shared")
                    zT = zpool.tile([P, KD, B], wdt, tag="zT")
                    if no_cc:
                        nc.sync.dma_start(zout[0, :, :], zin[:])
                    else:
                        nc.gpsimd.collective_compute(
                            "AllGather", mybir.AluOpType.bypass,
                            replica_groups=RG,
                            ins=[zin.opt()], outs=[zout.opt()])
                    zrb_eng = (nc.sync, nc.scalar, nc.gpsimd, nc.sync)
                    for i, c in enumerate(range(0, NC, 2)):
                        zrb_eng[i].dma_start(
                            zT[:, c:c + 2, :],
                            zout[c:c + 2].rearrange("c p b -> p c b"))

            z2 = locpool.tile([P, B], f32, tag="z2")
            nc.scalar.activation(z2[:], zloc[:], Square)
            psq = pspool.tile([1, B], f32, tag="psq")
            nc.tensor.matmul(psq[:], ones[:], z2[:], start=True, stop=True)
            sq_sb = locpool.tile([1, B], f32, tag="sqsb")
            nc.vector.tensor_copy(sq_sb[:], psq[:])
            nc.sync.dma_start(sq_d[:], sq_sb[:])

    nc.compile()
    _CACHED_NC[repeat] = nc
    return nc


def kernel(x, W0, b0, W1, b1, W2, b2, W3, b3):
    from concourse import bass_utils

    in_maps = _prep_inputs(x, W0, b0, W1, b1, W2, b2, W3, b3)
    nc = _build_module()
    res = bass_utils.run_bass_kernel_spmd(
        nc, in_maps, core_ids=list(range(NC)),
        trace=bool(int(os.environ.get("MAF_TRACE", "0"))))
    total = np.zeros(B, dtype=np.float64)
    for c in range(NC):
        total += res.results[c]["sq"][0].astype(np.float64)
    # device sq = sum over dims of (S*z)^2 -> divide by S^2
    out = 0.5 * total / (SCALE * SCALE) + 0.5 * DIM * LOG_2PI
    if res.exec_time_ns is not None:
        kernel.last_exec_time_ns = res.exec_time_ns
    return out.astype(F32)


kernel.last_exec_time_ns = None


# revision 21
# speedup vs baseline: 1.0066x; 1.0066x over previous
"""Trainium2 Bass kernel for the 4-step shift-only MAF (MADE) chain.

Strategy: tensor-parallel over the hidden/feature dims across 8 NeuronCores
(column-parallel for every layer), with activations kept transposed
[features, batch] so matmuls chain without transposes.  The inter-step
`z[:, ::-1]` permute is folded into the host-side weight prep (W0 rows /
W3 cols reversed for odd steps), so the device never flips.  After each
layer an AllGather (partition-axis concat) rebuilds the full activation.

v2 perf changes vs the plain bf16 baseline:
  - fp8e4 (e4m3) weights AND activations, x16 scaling on both (values
    stay in normal range; relu is positively homogeneous so the scale
    folds into the activation instruction's scale/bias).  Matmuls run
    in DoubleRow perf mode (2 k-tiles per instruction, 2x PE rate).
  - a dummy 0-dep AllGather issued at kernel start pulls the one-time
    collective entry barrier off the critical path (it overlaps the
    initial weight DMA + L0 compute instead of stalling the first real
    AllGather).
  - AllGather bounce traffic is spread across engine queues: gather
    results are read back per-rank (8 small contiguous DMAs on
    sync/vector instead of one big strided rearrange on sync), so
    matmul k-pairs chase individual rank landings via subtile deps;
    weight prefetch lives on gpsimd's queue; AG-input bounce writes on
    scalar's queue right behind the relu that produces them.

Device per-core program (SPMD, identical for all cores; per-core data
arrives via in_maps):
  z_loc [128,100] (f32, x16) and full zT [128,8,100] (fp8, x16) start
  as x.  Per step s: h0 = relu(W0e[s].T @ z) (2 psum m-tiles) -> AG ->
  h1 -> AG -> h2 -> AG -> shift = W3e[s].T @ h2; z_loc -= shift + b3;
  AG z (not on last step).  Finally out = ones.T @ (z_loc^2) per core
  -> [1,100]; host sums the 8 partials, divides by the scale^2, and
  adds the log(2pi) constant.
"""

import os
import sys

import numpy as np

for _p in ("/opt/trn_rl_repo", "/opt/trn_rl_repo/concourse"):
    if _p not in sys.path:
        sys.path.insert(0, _p)

B = 100
DIM = 1024
H = 2048
STEPS = 4
NC = 8
P = 128
KD = DIM // P   # 8 z k-tiles
KH = H // P     # 16 h k-tiles
MH = 2          # h m-tiles per core (256 local cols)
HL = H // NC    # 256
DL = DIM // NC  # 128
LOG_2PI = float(np.log(2.0 * np.pi))
F32 = np.float32

# compute dtype for weights / gathered activations ("float8e4" | "bfloat16")
WDTYPE = os.environ.get("MAF_WDTYPE", "float8e4")
SCALE = 16.0 if WDTYPE == "float8e4" else 1.0  # both weight and act scale


def _np_wdt():
    from concourse import mybir
    if WDTYPE == "float8e4":
        return mybir.dt.np(mybir.dt.float8e4)
    from ml_dtypes import bfloat16
    return bfloat16


def _made_mask(n_in, n_out, exclusive):
    d_in, d_out = n_in // DIM, n_out // DIM
    deg_in = np.arange(n_in) // d_in
    deg_out = np.arange(n_out) // d_out
    if exclusive:
        m = deg_out[None, :] > deg_in[:, None]
    else:
        m = deg_out[None, :] >= deg_in[:, None]
    return m.astype(F32)


def _prep_inputs(x, W0, b0, W1, b1, W2, b2, W3, b3):
    """Host-side: mask, fold flips, shard, scale, pre-arrange into SBUF
    layouts.  Returns in_maps: list of dicts, one per core."""
    M0 = _made_mask(DIM, H, True)
    M1 = _made_mask(H, H, False)
    M3 = _made_mask(H, DIM, False)

    xT = np.ascontiguousarray(x.T.astype(F32))              # [1024, 100]
    xt_arr = np.ascontiguousarray(
        xT.reshape(KD, P, B).transpose(1, 0, 2))            # [128, 8, 100]

    # Per-step effective (masked + flip-folded) weights
    W0e, W1e, W2e, W3e, b3e = [], [], [], [], []
    for s in range(STEPS):
        w0 = W0[s] * M0
        if s % 2 == 1:
            w0 = w0[::-1, :]
        w3 = W3[s] * M3
        b3s = b3[s]
        if s % 2 == 1:
            w3 = w3[:, ::-1]
            b3s = b3s[::-1]
        W0e.append(w0)
        W1e.append(W1[s] * M1)
        W2e.append(W2[s] * M1)
        W3e.append(w3)
        b3e.append(b3s)

    wdt = _np_wdt()
    S = SCALE
    in_maps = []
    for c in range(NC):
        hc = slice(HL * c, HL * (c + 1))
        dc = slice(DL * c, DL * (c + 1))
        w0c = np.stack([
            (W0e[s][:, hc] * S).reshape(KD, P, MH, P).transpose(1, 0, 2, 3)
            for s in range(STEPS)])                          # [4,128,8,2,128]
        w1c = np.stack([
            (W1e[s][:, hc] * S).reshape(KH, P, MH, P).transpose(1, 0, 2, 3)
            for s in range(STEPS)])                          # [4,128,16,2,128]
        w2c = np.stack([
            (W2e[s][:, hc] * S).reshape(KH, P, MH, P).transpose(1, 0, 2, 3)
            for s in range(STEPS)])
        w3c = np.stack([
            (W3e[s][:, dc] * S).reshape(KH, P, P).transpose(1, 0, 2)
            for s in range(STEPS)])                          # [4,128,16,128]
        # biases for relu layers are applied in the x{S} activation domain;
        # all four packed into one [P, 7] tensor per step (single DMA):
        # cols 0:2 = b0 (m0,m1), 2:4 = b1, 4:6 = b2, 6 = b3.
        ball = np.stack([
            np.concatenate([
                (b0[s][hc] * S).reshape(MH, P).T,
                (b1[s][hc] * S).reshape(MH, P).T,
                (b2[s][hc] * S).reshape(MH, P).T,
                (b3e[s][dc] * S).reshape(1, P).T,
            ], axis=1)
            for s in range(STEPS)])                          # [4, 128, 7]
        in_maps.append({
            "xt": np.ascontiguousarray((xt_arr * S).astype(wdt)),
            "xloc": np.ascontiguousarray(xT[dc, :] * S),     # [128, 100] f32
            "w0": np.ascontiguousarray(w0c.astype(wdt)),
            "w1": np.ascontiguousarray(w1c.astype(wdt)),
            "w2": np.ascontiguousarray(w2c.astype(wdt)),
            "w3": np.ascontiguousarray(w3c.astype(wdt)),
            "ball": np.ascontiguousarray(ball.astype(F32)),
        })
    return in_maps


_CACHED_NC = {}


def _build_module(repeat=1):
    """Build the SPMD module. repeat>1 runs the whole MAF body N times
    back-to-back (timing builds only; output is then meaningless)."""
    if repeat in _CACHED_NC:
        return _CACHED_NC[repeat]

    from concourse import bass, bacc, tile, mybir

    f32 = mybir.dt.float32
    is_fp8 = WDTYPE == "float8e4"
    wdt = mybir.dt.float8e4 if is_fp8 else mybir.dt.bfloat16
    KS = 2 if is_fp8 else 1          # k-tiles consumed per matmul
    PM = mybir.MatmulPerfMode.DoubleRow if is_fp8 else None
    INV_S = 1.0 / SCALE
    Relu = mybir.ActivationFunctionType.Relu
    Ident = mybir.ActivationFunctionType.Identity
    Square = mybir.ActivationFunctionType.Square
    RG = [list(range(NC))]
    no_cc = bool(int(os.environ.get("MAF_NO_CC", "0")))    # timing ablation
    # opt-in: a 0-dep warmup collective.  Measured on the axon pool it
    # LOSES ~15us (the CC stream serializes barrier -> dummy -> real AG).
    use_dummy = bool(int(os.environ.get("MAF_DUMMY", "0")))

    nc = bacc.Bacc("TRN2", target_bir_lowering=False, debug=False,
                   num_devices=NC)

    xt_d = nc.dram_tensor("xt", [P, KD, B], wdt, kind="ExternalInput")
    xloc_d = nc.dram_tensor("xloc", [P, B], f32, kind="ExternalInput")
    w0_d = nc.dram_tensor("w0", [STEPS, P, KD, MH, P], wdt, kind="ExternalInput")
    w1_d = nc.dram_tensor("w1", [STEPS, P, KH, MH, P], wdt, kind="ExternalInput")
    w2_d = nc.dram_tensor("w2", [STEPS, P, KH, MH, P], wdt, kind="ExternalInput")
    w3_d = nc.dram_tensor("w3", [STEPS, P, KH, P], wdt, kind="ExternalInput")
    ball_d = nc.dram_tensor("ball", [STEPS, P, 3 * MH + 1], f32,
                            kind="ExternalInput")
    sq_d = nc.dram_tensor("sq", [1, B], f32, kind="ExternalOutput")

    trace_sim = bool(int(os.environ.get("MAF_TRACE_SIM", "0")))
    with tile.TileContext(nc, trace_sim=trace_sim) as tc:
        with (
            # bufs=4: all four steps' weights prefetch at kernel start
            # (during the collective entry barrier, while HBM is idle),
            # so early-step bounce receipts don't contend with weight
            # streaming.  ~44KB/partition of SBUF, well within budget.
            tc.tile_pool(name="w01", bufs=4) as wpool,
            tc.tile_pool(name="hf", bufs=2) as hpool,
            tc.tile_pool(name="zp", bufs=2) as zpool,
            tc.tile_pool(name="loc", bufs=2) as locpool,
            tc.tile_pool(name="bia", bufs=4) as bpool,
            tc.tile_pool(name="cst", bufs=1) as cpool,
            tc.tile_pool(name="ps", bufs=4, space=bass.MemorySpace.PSUM) as pspool,
            tc.tile_pool(name="drb", bufs=2, space="DRAM") as dpool,
        ):
            if use_dummy and not no_cc:
                # 0-dependency warmup collective: absorbs the one-time
                # entry barrier while weights stream in.
                dmi = cpool.tile([P, 4], mybir.dt.int8, tag="dmi")
                nc.gpsimd.memset(dmi[:], 0)
                dum_in = dpool.tile([P, 4], mybir.dt.int8, tag="dmin", bufs=1)
                nc.gpsimd.dma_start(dum_in[:], dmi[:])
                dum_out = dpool.tile([NC, P, 4], mybir.dt.int8, tag="dmout",
                                     bufs=1)
                nc.gpsimd.collective_compute(
                    "AllGather", mybir.AluOpType.bypass, replica_groups=RG,
                    ins=[dum_in.opt()], outs=[dum_out.opt()])

            ones = cpool.tile([P, 1], f32, tag="ones")
            nc.gpsimd.memset(ones[:], 1.0)

            zT = zpool.tile([P, KD, B], wdt, tag="zT")
            nc.sync.dma_start(zT[:], xt_d[:])  # xt pre-arranged [p, c, b]
            zloc = zpool.tile([P, B], f32, tag="zloc")
            nc.sync.dma_start(zloc[:], xloc_d[:])

            def h_layer(w_t, b_t, rhsT, n_k, out_tag):
                """col-parallel hidden layer + AG; returns full hT tile."""
                kp = n_k // KS
                if len(rhsT.shape) == 4:
                    # hT [P, NC, MH, B]: pair j == rank j's block (fp8),
                    # or single k-tile (k//MH, k%MH) in bf16 mode.
                    if KS == 2:
                        rhs_j = lambda j: rhsT[:, j, :, :]
                    else:
                        rhs_j = lambda j: rhsT[:, j // MH, j % MH, :]
                else:
                    # zT [P, KD, B]
                    if KS == 2:
                        rhs_j = lambda j: rhsT[:, 2 * j:2 * j + 2, :]
                    else:
                        rhs_j = lambda j: rhsT[:, j, :]
                hloc = locpool.tile([P, MH, B], wdt, tag="hloc")
                agi = dpool.tile([P, MH, B], wdt, tag="agi")
                for m in range(MH):
                    ps = pspool.tile([P, B], f32, tag="ps")
                    for j in range(kp):
                        if KS == 2:
                            w_ap = w_t[:, 2 * j:2 * j + 2, m, :]
                        else:
                            w_ap = w_t[:, j, m, :]
                        nc.tensor.matmul(
                            ps[:], w_ap, rhs_j(j),
                            start=(j == 0), stop=(j == kp - 1), perf_mode=PM)
                    nc.scalar.activation(hloc[:, m, :], ps[:], Relu,
                                         bias=b_t[:, m:m + 1], scale=INV_S)
                    # bounce write per m-tile on SEPARATE queues: m0's HBM
                    # write+receipt overlaps m1's matmuls+relu, and m1's
                    # receipt (which gates the AG trigger) doesn't queue
                    # behind m0's on the same HWDGE ring.
                    weng = nc.scalar if m == 0 else nc.sync
                    weng.dma_start(agi[:, m, :], hloc[:, m, :],
                                   single_packet=True)
                ago = dpool.tile([NC, P, MH, B], wdt, tag="ago",
                                 addr_space="Shared")
                hT = hpool.tile([P, NC, MH, B], wdt, tag=out_tag)
                if no_cc:
                    nc.sync.dma_start(ago[0, :, :, :], agi[:])
                else:
                    nc.gpsimd.collective_compute(
                        "AllGather", mybir.AluOpType.bypass, replica_groups=RG,
                        ins=[agi.opt()], outs=[ago.opt()])
                # rank-pair contiguous readback on two queues; matmul k-pairs
                # chase individual pair landings via subtile deps.
                for c in range(0, NC, 2):
                    eng = nc.sync if c % 4 == 0 else nc.scalar
                    eng.dma_start(hT[:, c:c + 2, :, :],
                                  ago[c:c + 2].rearrange("c p m b -> p c m b"))
                return hT

            for it in range(STEPS * repeat):
                s = it % STEPS
                is_last = it == STEPS * repeat - 1
                w0t = wpool.tile([P, KD, MH, P], wdt, tag="w0")
                nc.gpsimd.dma_start(w0t[:], w0_d[s])
                w1t = wpool.tile([P, KH, MH, P], wdt, tag="w1")
                nc.gpsimd.dma_start(w1t[:], w1_d[s])
                w2t = wpool.tile([P, KH, MH, P], wdt, tag="w2")
                nc.gpsimd.dma_start(w2t[:], w2_d[s])
                w3t = wpool.tile([P, KH, P], wdt, tag="w3")
                nc.gpsimd.dma_start(w3t[:], w3_d[s])
                ballt = bpool.tile([P, 3 * MH + 1], f32, tag="ball")
                nc.gpsimd.dma_start(ballt[:], ball_d[s])
                b0t, b1t, b2t = (ballt[:, 2 * i:2 * i + MH] for i in range(3))
                b3t = ballt[:, 3 * MH:3 * MH + 1]

                h0T = h_layer(w0t, b0t, zT, KD, "h0T")
                h1T = h_layer(w1t, b1t, h0T, KH, "h1T")
                h2T = h_layer(w2t, b2t, h1T, KH, "h2T")

                ps3 = pspool.tile([P, B], f32, tag="ps")
                for j in range(KH // KS):
                    if KS == 2:
                        nc.tensor.matmul(ps3[:], w3t[:, 2 * j:2 * j + 2, :],
                                         h2T[:, j, :, :],
                                         start=(j == 0),
                                         stop=(j == KH // KS - 1),
                                         perf_mode=PM)
                    else:
                        nc.tensor.matmul(ps3[:], w3t[:, j, :],
                                         h2T[:, j // MH, j % MH, :],
                                         start=(j == 0),
                                         stop=(j == KH - 1))
                # sh = shift*S + b3*S (still in the xS domain)
                sh = locpool.tile([P, B], f32, tag="sh")
                nc.scalar.activation(sh[:], ps3[:], Ident,
                                     bias=b3t[:, 0:1], scale=INV_S)
                if not is_last:
                    # fp8 AG input first (critical path), f32 update after
                    # (overlaps the collective).
                    zlb = locpool.tile([P, B], wdt, tag="zlb")
                    nc.vector.tensor_sub(zlb[:], zloc[:], sh[:])
                zloc2 = zpool.tile([P, B], f32, tag="zloc")
                nc.vector.tensor_sub(zloc2[:], zloc[:], sh[:])
                zloc = zloc2

                if not is_last:
                    zin = dpool.tile([P, B], wdt, tag="zin")
                    # sync queue: issues right behind the vector sub that
                    # produces zlb, without queuing behind scalar's acts.
                    nc.sync.dma_start(zin[:], zlb[:], single_packet=True)
                    zout = dpool.tile([NC, P, B], wdt, tag="zout",
                                      addr_space="Shared")
                    zT = zpool.tile([P, KD, B], wdt, tag="zT")
                    if no_cc:
                        nc.sync.dma_start(zout[0, :, :], zin[:])
                    else:
                        nc.gpsimd.collective_compute(
                            "AllGather", mybir.AluOpType.bypass,
                            replica_groups=RG,
                            ins=[zin.opt()], outs=[zout.opt()])
                    for c in range(0, NC, 2):
                        eng = nc.sync if c % 4 == 0 else nc.scalar
                        eng.dma_start(zT[:, c:c + 2, :],
                                      zout[c:c + 2].rearrange("c p b -> p c b"))

            z2 = locpool.tile([P, B], f32, tag="z2")
            nc.scalar.activation(z2[:], zloc[:], Square)
            psq = pspool.tile([1, B], f32, tag="psq")
            nc.tensor.matmul(psq[:], ones[:], z2[:], start=True, stop=True)
            sq_sb = locpool.tile([1, B], f32, tag="sqsb")
            nc.vector.tensor_copy(sq_sb[:], psq[:])
            nc.sync.dma_start(sq_d[:], sq_sb[:])

    nc.compile()
    _CACHED_NC[repeat] = nc
    return nc


def kernel(x, W0, b0, W1, b1, W2, b2, W3, b3):
    from concourse import bass_utils

    in_maps = _prep_inputs(x, W0, b0, W1, b1, W2, b2, W3, b3)
    nc = _build_module()
    res = bass_utils.run_bass_kernel_spmd(
        nc, in_maps, core_ids=list(range(NC)),
        trace=bool(int(os.environ.get("MAF_TRACE", "0"))))
    total = np.zeros(B, dtype=np.float64)
    for c in range(NC):
        total += res.results[c]["sq"][0].astype(np.float64)
    # device sq = sum over dims of (S*z)^2 -> divide by S^2
    out = 0.5 * total / (SCALE * SCALE) + 0.5 * DIM * LOG_2PI
    if res.exec_time_ns is not None:
        kernel.last_exec_time_ns = res.exec_time_ns
    return out.astype(F32)


kernel.last_exec_time_ns = None
